# revision 27
# baseline (speedup 1.0000x reference)
"""Baichuan attention on 8 Trainium2 NeuronCores — tensor-parallel over heads.

Sharding: core c computes heads [4c, 4c+4): its slice of the fused QKV
projection, attention for those heads, then 1/8 of o_proj's output columns
after an AllGather of the per-core context slices (moves 4MB/rank instead of
a 32MB AllReduce of partial sums; mathematically identical to the module's
world_size logic).

Schedule: attention blocks interleave into the QKV stream at the earliest
point their k/v/q tiles exist, so every AllGather trigger fires as early
as structurally possible:

  half0-QKV | att(0) att(1) | qk-sb0 v[8-11] att(2) qk-sb1(rev) v[12-15]
  att(3) | op(0..3)

The serial CC-core gather chain (2MB head-pair ops, ~17-30us each, 35-50us
for the first-after-idle op which absorbs inter-rank drift) then finishes
~60us before the last o_proj block needs its ct. The reversed-ot second qk
pass reuses the 3 w tiles still live in the pool, and o_proj's contraction
is split into the two gather-pair phases (w_o rows host-permuted so tiles
0-15 = pair0 of all ranks) so op(b) can start on its pp0 gather while pp1
is in flight.

DMA-queue discipline (single hw queue per engine, in-order, head-of-line
blocking): ct staging physically reuses the x1/w1 SBUF region, so its
descriptors wait on the last x read (~465us) — ctpool opens before owpool
so ct0 lands on the earliest-freed chunk region, emission order matches
consumption (ct0, wo0-1, ct1, wo2-3, ops, ct2, ct3), and 32 ct bufs double-
buffer so staging never waits on pool frees. Inputs are host-retiled
partition-major ([P, blk, t, inner]) so each chunk DMA is one contiguous
8KB run per partition — the naive [H, S] layouts expanded to 1KB/256B
descriptors and burned ~40us of engine-serialized descriptor generation at
startup. Every chunk is its OWN tile (reads wait all writers of a tile, so
monolithic tiles serialized the first matmul behind whole-block arrival).
x-sb1 rides the scalar engine's hwdge queue in parallel with sync; tri
defers past the startup window; wv defers to ot==7; x1/w1 prefetch before
att(0) so the 8MB stream hides behind attention PE work. Startup remains
supply-paced (~180-300GB/s while all 8 cores pull through shared HBM
ports): ~15us of PE idle is structural.

Causal structure: diagonal score tiles are narrowed to their live columns
(moving width 512-128j) and masked with a single resident 128x128 triangular
constant instead of a 4MB mask DMA. Score tiles are computed in pairs into
2-bank PSUM tiles so one exp activation covers both; av matmuls lag scores
by TWO slots (ps2 is freed by the exp read, not by av, so depth-2 costs no
extra PSUM). Softmax denominators: exp tiles accumulate on the vector
engine into an fp16 SBUF tile, ONE 512-wide fp16 ones-matmul per head does
the cross-partition sum, reciprocal_approx_fast computes 1/den. PSUM:
scores 2x2 banks + attention-out 1 + rowsum 1 + (qkv 2 | o_proj 2) = 8.
out_cols written fp16 (host casts back; halves output traffic).

Matmul operands are fp16 (1 cyc/row) with fp32 PSUM accumulation.
LDWEIGHTS is fully hidden by the PE's reorder window; back-to-back MMs
issue at exactly N cycles. The chip's power manager holds the PE at
K=13/16 of 2.4GHz (~1.95GHz, 263ns per 512-wide MM) for any sustained
stream — brief 216ns/2.4GHz bursts appear only in the first ~30us — so
the streaming floor is 1.196M cycles ~= 613us. Measured 645-651us
(baseline 719.8us): ~15us startup supply + ~9us preamble + ~6us tail +
~5us att/op micro-bubbles. fp8 was ruled out: random-sign contractions
preserve operand quantization error (~2-3% rel for e4m3) with no sqrt(N)
averaging, blowing the 2e-2 budget.
"""

import numpy as np

import concourse.bacc as bacc
import concourse.bass_isa as bass_isa
import concourse.mybir as mybir
import concourse.tile as tile
from concourse.bass_utils import run_bass_kernel_spmd

F32 = mybir.dt.float32

N_CORES = 8
NUM_HEADS = 32
HEAD_DIM = 128
P = 128          # SBUF partitions / PE contraction tile
SQ = 512         # s_q block width (PSUM bank = 512 fp32)
MM_MODE = "f16"  # 'f16' | 'f32' (operand dtype for matmuls)

_CACHE: dict = {}


def _mm_dtype(mode):
    return {"f16": mybir.dt.float16, "f32": F32}[mode]


def build(S, H, mode=MM_MODE):
    MD = _mm_dtype(mode)
    hpc = NUM_HEADS // N_CORES          # heads per core
    dpc = hpc * HEAD_DIM                # per-core slice of the hidden dim
    n_ht = H // P                       # contraction tiles for QKV/o_proj
    n_qk = 2 * dpc // P                 # q+k output tiles
    n_sq = S // SQ                      # s_q blocks
    n_st = S // P                       # s_k tiles
    scale = 1.0 / np.sqrt(np.float32(HEAD_DIM))
    s_half = S // 2
    sb_per_half = s_half // SQ
    diag_per_b = SQ // P                # diagonal k-tiles per s_q block

    nc = bacc.Bacc("TRN2", target_bir_lowering=False, debug=False,
                   num_devices=N_CORES)

    # inputs are host-retiled partition-major so every DMA chunk is one
    # contiguous multi-KB run per partition: naive [H, S]-style layouts
    # expand to 1KB/256B descriptors (1024-4096 per chunk) and the sync
    # engine pays ~4-8ns per descriptor — x0's load alone burned ~40us of
    # engine-serialized descriptor pushing
    x5 = nc.dram_tensor("x5", [P, S // SQ, n_ht, SQ], MD,
                        kind="ExternalInput")
    wqk5 = nc.dram_tensor("wqk5", [P, n_qk, n_ht, P], MD,
                          kind="ExternalInput")
    wv5 = nc.dram_tensor("wv5", [P, n_ht, dpc], MD, kind="ExternalInput")
    tri = nc.dram_tensor("tri", [P, P], F32, kind="ExternalInput")
    wo5 = nc.dram_tensor("wo5", [P, n_ht, dpc], MD, kind="ExternalInput")
    out_cols = nc.dram_tensor("out_cols", [S, dpc], MD,
                              kind="ExternalOutput")

    # AllGather in head-pair chunks: gat[b][pp] holds local heads
    # {2pp, 2pp+1} for s_q block b; ct[b][pp] gathers those pairs from all
    # ranks. o_proj consumes them against host-permuted w_o rows.
    gat_b = [[nc.dram_tensor(f"gat_{b}_{pp}", [dpc // 2, SQ], MD)
              for pp in range(2)] for b in range(n_sq)]
    ct_b = [[nc.dram_tensor(f"ct_{b}_{pp}", [H // 2, SQ], MD,
                            addr_space="Shared") for pp in range(2)]
            for b in range(n_sq)]



    with tile.TileContext(nc) as tc:
        with (
            tc.tile_pool(name="consts", bufs=1) as cpool,
            tc.tile_pool(name="span", bufs=1) as span,
            tc.tile_pool(name="qkv_wv", bufs=1) as wvpool,
            tc.tile_pool(name="at_exp", bufs=3) as epool,
            tc.tile_pool(name="at_out", bufs=2) as opool,
            tc.tile_pool(name="at_r", bufs=2) as rpool,
        ):
            ones_s = cpool.tile([P, P], F32, tag="ones_s")
            nc.gpsimd.memset(ones_s[:], 1.0)
            ones_f = cpool.tile([P, P], MD, tag="ones_f")
            nc.vector.tensor_copy(ones_f[:], ones_s[:])
            tri_t = cpool.tile([P, P], F32, tag="tri")

            # v ([s_k, d] natural, all heads) and q/k (transposed, all heads)
            # live in SBUF for the whole kernel; QKV evictions write them
            # directly (no DRAM bounce)
            v_sb = span.tile([P, n_st, dpc], MD, tag="v")
            qk_all = span.tile([P, n_qk, S], MD, tag="qk")
            wv_sb = wvpool.tile([P, n_ht, dpc], MD, tag="wv")

            # x arrives in 8-t-tile chunks: batching descriptors keeps the
            # sync queue's ~0.7us/descriptor issue rate off the critical
            # path.
            XCH = 8

            NXC = 32 // XCH     # chunk tiles per sb block

            # each 8-t chunk is its OWN tile: readers then depend on just
            # their chunk's DMA, not the whole 4MB block (tile reads wait
            # ALL writers of the tile, so a monolithic x tile serialized
            # the first matmul behind the full block's arrival)
            def load_x(half, xpool):
                xq = [[xpool.tile([P, XCH, SQ], MD, tag="x", bufs=2 * NXC,
                                  name="x_tile") for _ in range(NXC)]
                      for _ in range(sb_per_half)]
                for sb in range(sb_per_half):
                    for c in range(NXC):
                        cb = half * sb_per_half + sb
                        nc.sync.dma_start(
                            xq[sb][c][:],
                            x5.ap()[:, cb, c * XCH:(c + 1) * XCH, :])
                return xq

            def xsl(xq_sb, t):
                ch = n_ht // len(xq_sb)
                return xq_sb[t // ch][:, t % ch, :]

            def wsl(w_ot, t):
                if isinstance(w_ot, list):
                    ch = n_ht // len(w_ot)
                    return w_ot[t // ch][:, t % ch, :]
                return w_ot[:, t, :]

            # q/k projection for the given sb blocks of one half, output
            # transposed into resident qk_all. ot_order lets the second
            # half1 pass run reversed so it reuses the 3 still-cached w
            # tiles instead of reloading them.
            def qkv_qk(half, xq, w_tiles, wpool, pspool, sbs,
                       ot_order=None):
                for ot in (ot_order or range(n_qk)):
                    if ot not in w_tiles:
                        w_tiles[ot] = wpool.tile([P, n_ht, P], MD,
                                                 tag="w", name="w_tile")
                        nc.sync.dma_start(
                            w_tiles[ot][:], wqk5.ap()[:, ot, :, :])
                    w_ot = w_tiles[ot]
                    # wv is first needed ~134us in (the v phase); issuing it
                    # at ot==7 keeps its 4MB out of the bandwidth-starved
                    # startup window
                    if half == 0 and ot == 7:
                        nc.sync.dma_start(wv_sb[:], wv5.ap()[:, :, :])
                    if half == 0:
                        # t-outer with both quarters' psums open: the first
                        # ot pass consumes x chunks in DMA-arrival order
                        # instead of draining quarter 0 first, which halves
                        # the startup underrun while x is still streaming
                        # in. Needs 4 psum bufs, so half1 (2 bufs, x fully
                        # prefetched behind att(0)/att(1)) keeps the
                        # sb-inner form.
                        ps_q = [pspool.tile([P, SQ], F32, tag="qkv",
                                            name=f"ps_q{sb}")
                                for sb in sbs]
                        for t in range(n_ht):
                            for si, sb in enumerate(sbs):
                                nc.tensor.matmul(
                                    ps_q[si][:],
                                    wsl(w_ot, t),
                                    xsl(xq[sb], t),
                                    start=(t == 0), stop=(t == n_ht - 1))
                        for si, sb in enumerate(sbs):
                            # fold the softmax scale into q at eviction
                            mul = scale if ot < dpc // P else 1.0
                            lo = half * s_half + sb * SQ
                            nc.scalar.mul(qk_all[:, ot, lo:lo + SQ],
                                          ps_q[si][:], mul)
                    else:
                        for sb in sbs:
                            ps = pspool.tile([P, SQ], F32, tag="qkv")
                            for t in range(n_ht):
                                nc.tensor.matmul(
                                    ps[:],
                                    wsl(w_ot, t),
                                    xsl(xq[sb], t),
                                    start=(t == 0), stop=(t == n_ht - 1))
                            mul = scale if ot < dpc // P else 1.0
                            lo = half * s_half + sb * SQ
                            nc.scalar.mul(qk_all[:, ot, lo:lo + SQ],
                                          ps[:], mul)

            # v projection for the given local s-tiles of one half:
            # psum [s=128, dpc] accumulated over h-tiles
            def qkv_v(half, xq, pspool, stis):
                for sti in stis:
                    st_g = half * (s_half // P) + sti
                    sb, off = (sti * P) // SQ, (sti * P) % SQ
                    ps_v = pspool.tile([P, dpc], F32, tag="qkv")
                    for t in range(n_ht):
                        xch = n_ht // len(xq[sb])
                        nc.tensor.matmul(
                            ps_v[:],
                            xq[sb][t // xch][:, t % xch, off:off + P],
                            wv_sb[:, t, :],
                            start=(t == 0), stop=(t == n_ht - 1))
                    nc.vector.tensor_copy(v_sb[:, st_g, :], ps_v[:])

            with (
                tc.tile_pool(name="qkv_x0", bufs=1) as xpool0,
                tc.tile_pool(name="qkv_w0", bufs=3) as wpool0,
                tc.tile_pool(name="qkv_ps0", bufs=4, space="PSUM") as psp0,
            ):
                # startup ordering: the t-outer first ot pass consumes
                # (w0 t-chunk, x sb0 chunk, x sb1 chunk) groups in order, so
                # issue the DMAs in exactly that order — the first matmul
                # then waits on 2.25MB, not on all of x + w.
                XC0 = XCH // 2      # finer startup chunks: first matmul
                NX0 = n_ht // XC0   # waits on ~1.1MB, not 2.25MB
                w0c = [wpool0.tile([P, XC0, P], MD, tag="w0c",
                                   name="w0c") for _ in range(NX0)]
                w0_tiles = {0: w0c}
                xq0 = [[xpool0.tile([P, XC0, SQ], MD, tag="x",
                                    bufs=2 * NX0, name="x_tile")
                        for _ in range(NX0)] for _ in range(sb_per_half)]
                # sb1's chunks ride the scalar engine's separate hardware
                # DMA queue: chunk pairs arrive in parallel and the
                # descriptor issue cost is split across two engines
                for c in range(NX0):
                    nc.sync.dma_start(
                        w0c[c][:], wqk5.ap()[:, 0, c * XC0:(c + 1) * XC0, :])
                    nc.sync.dma_start(
                        xq0[0][c][:], x5.ap()[:, 0, c * XC0:(c + 1) * XC0, :])
                    nc.scalar.dma_start(
                        xq0[1][c][:], x5.ap()[:, 1, c * XC0:(c + 1) * XC0, :])
                # tri is first read at att(0), ~215us in — keep its 64KB of
                # 512B-packet traffic out of the startup window
                nc.scalar.dma_start(tri_t[:], tri.ap()[:, :])
                qkv_qk(0, xq0, w0_tiles, wpool0, psp0, [0, 1])
                qkv_v(0, xq0, psp0, range(s_half // P))



            # attention PSUM pools: pss 2x2 banks + out 1 + row 1 = 6 banks,
            # leaving 2 for the half1 QKV pool / later the o_proj pool
            with tc.tile_pool(name="at_ps", bufs=1, space="PSUM") as aps:

                # ======== attention for s_q block b (4 local heads) ========
                # scores are computed transposed (scoresT[k, q]) so the PE
                # contraction dim sits on partitions for every matmul.
                # Diagonal tiles are narrowed to columns [128j, 512) and get
                # the triangular mask strip added in place. Tiles are
                # processed in slots of two; exp of slot k runs while the PE
                # streams slot k+1's scores, and av/rowsum of slot k follow —
                # steady state has no PE bubble.
                def att_block(b):
                    q_lo = b * SQ
                    n_full = b * diag_per_b
                    slots = []
                    for g in range(n_full // 2):
                        slots.append([(2 * g, 0), (2 * g + 1, 0)])
                    for g in range(diag_per_b // 2):
                        slots.append([(n_full + 2 * g, 2 * g),
                                      (n_full + 2 * g + 1, 2 * g + 1)])
                    n_mm = n_full + diag_per_b
                    for h in range(hpc):
                        ps_o = aps.tile([P, SQ], F32, tag="out",
                                        name="ps_o")
                        # softmax denominator: accumulate exp tiles on the
                        # vector engine (fp16 adds into an SBUF fp32
                        # accumulator), cross-partition-sum via one
                        # ones-matmul per head
                        acc = rpool.tile([P, SQ], MD,
                                         tag="acc", name="acc")
                        mi = 0

                        def emit_av(pend):
                            nonlocal mi
                            slot, ex2 = pend
                            for ii, (t, j) in enumerate(slot):
                                lo = P * j
                                first, last = mi == 0, mi == n_mm - 1
                                nc.tensor.matmul(
                                    ps_o[:, lo:SQ],
                                    v_sb[:, t, h * P:(h + 1) * P],
                                    ex2[:, ii, lo:SQ],
                                    start=first, stop=last,
                                    skip_group_check=True)
                                if first:
                                    # first tile is always full-width
                                    nc.vector.tensor_copy(
                                        acc[:], ex2[:, ii, :])
                                else:
                                    nc.vector.tensor_add(
                                        acc[:, lo:SQ], acc[:, lo:SQ],
                                        ex2[:, ii, lo:SQ])
                                mi += 1

                        # av/rowsum lag scores by TWO slots: ps2 is freed by
                        # the exp read (not by av), so depth-2 costs no extra
                        # PSUM and the scores->exp->av chain latency is fully
                        # hidden even at block starts.
                        pend = []
                        for slot in slots:
                            ps2 = aps.tile([P, 2, SQ], F32, tag="pss",
                                           bufs=2, name="ps2")
                            ex2 = epool.tile([P, 2, SQ], MD, tag="exp",
                                             name="ex2")
                            is_diag = slot[0][0] >= n_full
                            for ii, (t, j) in enumerate(slot):
                                lo = P * j
                                nc.tensor.matmul(
                                    ps2[:, ii, lo:SQ],
                                    qk_all[:, hpc + h, t * P:(t + 1) * P],
                                    qk_all[:, h, q_lo + lo:q_lo + SQ],
                                    start=True, stop=True)
                                if is_diag:
                                    nc.vector.tensor_add(
                                        ps2[:, ii, lo:lo + P],
                                        ps2[:, ii, lo:lo + P], tri_t[:])
                            if is_diag:
                                for ii, (t, j) in enumerate(slot):
                                    lo = P * j
                                    nc.scalar.activation(
                                        ex2[:, ii, lo:SQ], ps2[:, ii, lo:SQ],
                                        mybir.ActivationFunctionType.Exp)
                            else:
                                nc.scalar.activation(
                                    ex2[:, :, :], ps2[:, :, :],
                                    mybir.ActivationFunctionType.Exp)
                            pend.append((slot, ex2))
                            if len(pend) > 2:
                                emit_av(pend.pop(0))
                        for p in pend:
                            emit_av(p)

                        # cross-partition sum of the exp accumulator in ONE
                        # 512-wide fp16 ones-matmul (f32r ran as fp32-HIGH,
                        # ~694ns + exposed fp32 LDW, ~8us of PE across the
                        # 16 head-blocks; fp16 accumulation of the <=2048-
                        # term denominator adds only ~1e-3 rel error)
                        ps_row = aps.tile([P, SQ], F32, tag="row",
                                          name="ps_row")
                        nc.tensor.matmul(
                            ps_row[:], ones_f[:], acc[:],
                            start=True, stop=True)
                        recip = rpool.tile([P, SQ], F32, tag="recip",
                                           name="recip")
                        nc.vector.reciprocal_approx_fast(recip[:], ps_row[:])
                        ob = opool.tile([P, SQ], MD, tag="ob", name="ob")
                        nc.vector.tensor_mul(ob[:], ps_o[:], recip[:])
                        nc.sync.dma_start(
                            gat_b[b][h // 2].ap()[(h % 2) * P:
                                                  (h % 2 + 1) * P, :], ob[:])
                        if h % 2 == 1:
                            nc.gpsimd.collective_compute(
                                "AllGather", mybir.AluOpType.bypass,
                                replica_groups=[list(range(N_CORES))],
                                ins=[gat_b[b][h // 2].ap().opt()],
                                outs=[ct_b[b][h // 2].ap().opt()])

                # att(0)+att(1) fit between the QKV halves: they need only
                # first-half k/v/q, and their ~26us of PE work hides the
                # half1 x/w prefetch (issued first, below — the x1 DMAs
                # have no dependencies and stream during qkv0's tail).
                # att(2)/att(3) interleave INTO half1 right after the k/v
                # tiles they need exist, so their gathers fire at ~370/~505
                # instead of after the whole half — the serial CC-core
                # chain (~20-40us per 2MB op) then finishes well before the
                # last o_proj block needs its ct.
                with (
                    tc.tile_pool(name="qkv_x1", bufs=1) as xpool1,
                    tc.tile_pool(name="qkv_w1", bufs=3) as wpool1,
                    tc.tile_pool(name="qkv_ps1", bufs=2,
                                 space="PSUM") as psp1,
                ):
                    w1_tiles = {0: wpool1.tile([P, n_ht, P], MD, tag="w",
                                               name="w_tile")}
                    nc.sync.dma_start(w1_tiles[0][:],
                                      wqk5.ap()[:, 0, :, :])
                    xq1 = load_x(1, xpool1)
                    att_block(0)
                    att_block(1)
                    qkv_qk(1, xq1, w1_tiles, wpool1, psp1, [0])
                    qkv_v(1, xq1, psp1, range(0, s_half // P // 2))
                    att_block(2)
                    # reversed ot: w7/w6/w5 are still live in the 3-buf
                    # pool from the sb0 pass — no reload for them, and the
                    # rotation's reuse-dependencies naturally pace the
                    # remaining reloads one ot ahead of consumption
                    w1b = {ot: w1_tiles[ot] for ot in (5, 6, 7)}
                    qkv_qk(1, xq1, w1b, wpool1, psp1, [1],
                           ot_order=range(n_qk - 1, -1, -1))
                    qkv_v(1, xq1, psp1, range(s_half // P // 2,
                                              s_half // P))
                    att_block(3)

                with (
                    tc.tile_pool(name="op_ct", bufs=32) as ctpool,
                    tc.tile_pool(name="op_w", bufs=1) as owpool,
                    tc.tile_pool(name="op_stage", bufs=4) as ospool,
                    tc.tile_pool(name="op_ps", bufs=2, space="PSUM") as opsp,
                ):
                    # wo in 4 chunk tiles interleaved with ct staging
                    # (all of it waits the last x1 read ~465us; op(0)'s
                    # first matmuls need only woc0 + ct0-pp0, so interleave
                    # in consumption order instead of serializing 4MB of wo
                    # ahead of 8MB of ct)
                    wo_sb = [owpool.tile([P, XCH, dpc], MD, tag="wo",
                                         bufs=NXC, name="woc")
                             for _ in range(NXC)]
                    def prefetch_ct(b):
                        # 2 t-tiles per descriptor: halves the post-gather
                        # issue serialization on the sync queue
                        cts = []
                        for pp in range(2):
                            ct_t = ct_b[b][pp].ap().rearrange(
                                "(t p) s -> p t s", p=P)
                            for t2 in range(n_ht // 4):
                                c_t = ctpool.tile([P, 2, SQ], MD, tag="ct")
                                nc.sync.dma_start(
                                    c_t[:], ct_t[:, 2 * t2:2 * t2 + 2, :])
                                cts.append(c_t)
                        return cts

                    def emit_oproj(b, cts):
                        # contraction split into pp phases (w_o rows are
                        # host-permuted so tiles 0-15 = pair0 of all ranks,
                        # 16-31 = pair1): a block can start on its pp0
                        # gather while pp1's is still in flight — op(3)'s
                        # pp1 lands only ~8us before it's needed
                        hht = n_ht // 2
                        for sp in range(SQ // P // 2):
                            pss = [opsp.tile([P, dpc], F32, tag="op",
                                             name="op_ps")
                                   for _ in range(2)]
                            for ph in range(2):
                                for si in range(2):
                                    st = 2 * sp + si
                                    for t in range(ph * hht,
                                                   (ph + 1) * hht):
                                        nc.tensor.matmul(
                                            pss[si][:],
                                            cts[t // 2][:, t % 2,
                                                        st * P:(st + 1) * P],
                                            wo_sb[t // XCH][:, t % XCH, :],
                                            start=(t == 0),
                                            stop=(t == n_ht - 1))
                                    if ph == 1:
                                        ob = ospool.tile(
                                            [P, dpc], MD, tag="ostage",
                                            name="ostage")
                                        nc.scalar.copy(ob[:], pss[si][:])
                                        nc.sync.dma_start(
                                            out_cols.ap()[
                                                b * SQ + st * P:
                                                b * SQ + (st + 1) * P, :],
                                            ob[:])

                    # ct staging physically reuses the x1/w1 SBUF region,
                    # so its descriptors can start only once the last x
                    # read (v[12-15], ~465us) completes — by then gathers
                    # g0..g2 are long done and the 32-buf pool double-
                    # buffers: ct0+ct1 stage back-to-back before op(0)'s
                    # PE arrival, ct2/ct3 into the halves freed by
                    # op(0)/op(1).
                    def wo_chunk(c):
                        nc.sync.dma_start(
                            wo_sb[c][:],
                            wo5.ap()[:, c * XCH:(c + 1) * XCH, :])

                    cts0 = prefetch_ct(0)
                    wo_chunk(0)
                    wo_chunk(1)
                    cts1 = prefetch_ct(1)
                    wo_chunk(2)
                    wo_chunk(3)
                    emit_oproj(0, cts0)
                    cts2 = prefetch_ct(2)
                    emit_oproj(1, cts1)
                    cts3 = prefetch_ct(3)
                    emit_oproj(2, cts2)
                    emit_oproj(3, cts3)

    nc.compile()
    return nc


def _tile5(a2d, inner):
    """[H, O] -> partition-major tiles [P, O//inner, H//P, inner]:
    per (partition, outer-block) the (t, inner) plane is contiguous, so
    chunk DMAs become one multi-KB descriptor per partition."""
    H, O = a2d.shape
    a = a2d.reshape(H // P, P, O // inner, inner)
    return np.ascontiguousarray(a.transpose(1, 2, 0, 3))


def make_in_maps(hidden_states, attention_mask, w_pack, w_o):
    B, S, H = hidden_states.shape
    hpc = NUM_HEADS // N_CORES
    dpc = hpc * HEAD_DIM
    np_md = mybir.dt.np(_mm_dtype(MM_MODE))
    xT = hidden_states[0].T.astype(np_md)
    x5 = _tile5(xT, SQ)
    # triangular mask strip for diagonal score tiles (scoresT layout:
    # rows=s_k, cols=s_q; masked where k > q -> strictly lower triangle)
    tri = np.tril(np.full((P, P), np.finfo(np.float32).min,
                          dtype=np.float32), k=-1)
    # w_o rows permuted to match the head-pair AllGather layout:
    # [pp][rank][head-in-pair] blocks of 128
    perm = np.concatenate(
        [np.arange(128 * (4 * r + 2 * pp + hh),
                   128 * (4 * r + 2 * pp + hh) + 128)
         for pp in (0, 1) for r in range(N_CORES) for hh in (0, 1)])
    in_maps = []
    for c in range(N_CORES):
        sl = slice(c * dpc, (c + 1) * dpc)
        wqk_c = np.concatenate(
            [w_pack[0 * H:1 * H][sl], w_pack[1 * H:2 * H][sl]], axis=0)
        woT_c = w_o[sl].T[perm]
        in_maps.append({
            "x5": x5,
            "wqk5": _tile5(wqk_c.T.astype(np_md), P),
            "wv5": _tile5(w_pack[2 * H:3 * H][sl].T.astype(np_md), dpc),
            "tri": tri,
            "wo5": _tile5(woT_c.astype(np_md), dpc),
        })
    return in_maps, tri


def kernel(hidden_states, attention_mask, w_pack, w_o):
    B, S, H = hidden_states.shape
    assert B == 1 and H == NUM_HEADS * HEAD_DIM
    assert S % (2 * SQ) == 0

    # the kernel hardcodes the causal structure; verify the mask matches
    mask = np.asarray(np.broadcast_to(attention_mask, (1, 1, S, S))[0, 0],
                      dtype=np.float32)
    assert np.all(np.tril(mask) == 0.0), "mask must be causal"
    assert np.all(mask[np.triu_indices(S, 1)] <= -1e30), "mask must be causal"

    in_maps, _ = make_in_maps(hidden_states, attention_mask, w_pack, w_o)

    key = (S, H, MM_MODE)
    if key not in _CACHE:
        _CACHE[key] = build(S, H, MM_MODE)
    nc = _CACHE[key]

    res = run_bass_kernel_spmd(nc, in_maps, core_ids=list(range(N_CORES)))
    out = np.concatenate(
        [res.results[c]["out_cols"] for c in range(N_CORES)], axis=1)
    return out.reshape(1, S, H).astype(np.float32)


# revision 28
# speedup vs baseline: 1.0062x; 1.0062x over previous
"""Baichuan attention on 8 Trainium2 NeuronCores — tensor-parallel over heads.

Sharding: core c computes heads [4c, 4c+4): its slice of the fused QKV
projection, attention for those heads, then 1/8 of o_proj's output columns
after an AllGather of the per-core context slices (moves 4MB/rank instead of
a 32MB AllReduce of partial sums; mathematically identical to the module's
world_size logic).

Schedule: attention blocks interleave into the QKV stream at the earliest
point their k/v/q tiles exist, so every AllGather trigger fires as early
as structurally possible:

  half0-QKV | att(0) att(1) | qk-sb0 v[8-11] att(2) qk-sb1(rev) v[12-15]
  att(3) | op(0..3)

The serial CC-core gather chain (2MB head-pair ops, ~17-30us each, 35-50us
for the first-after-idle op which absorbs inter-rank drift) then finishes
~60us before the last o_proj block needs its ct. The reversed-ot second qk
pass reuses the 3 w tiles still live in the pool, and o_proj's contraction
is split into the two gather-pair phases (w_o rows host-permuted so tiles
0-15 = pair0 of all ranks) so op(b) can start on its pp0 gather while pp1
is in flight.

DMA-queue discipline (single hw queue per engine, in-order, head-of-line
blocking): ct staging physically reuses the x1/w1 SBUF region, so its
descriptors wait on the last x read (~465us) — ctpool opens before owpool
so ct0 lands on the earliest-freed chunk region, emission order matches
consumption (ct0, wo0-1, ct1, wo2-3, ops, ct2, ct3), and 32 ct bufs double-
buffer so staging never waits on pool frees. Inputs are host-retiled
partition-major ([P, blk, t, inner]) so each chunk DMA is one contiguous
8KB run per partition — the naive [H, S] layouts expanded to 1KB/256B
descriptors and burned ~40us of engine-serialized descriptor generation at
startup. Every chunk is its OWN tile (reads wait all writers of a tile, so
monolithic tiles serialized the first matmul behind whole-block arrival).
x-sb1 rides the scalar engine's hwdge queue in parallel with sync; tri
defers past the startup window; wv defers to ot==7; x1/w1 prefetch before
att(0) so the 8MB stream hides behind attention PE work. Startup remains
supply-paced (~180-300GB/s while all 8 cores pull through shared HBM
ports): ~15us of PE idle is structural.

Causal structure: diagonal score tiles are narrowed to their live columns
(moving width 512-128j) and masked with a single resident 128x128 triangular
constant instead of a 4MB mask DMA. Score tiles are computed in pairs into
2-bank PSUM tiles so one exp activation covers both; av matmuls lag scores
by TWO slots (ps2 is freed by the exp read, not by av, so depth-2 costs no
extra PSUM). Softmax denominators: exp tiles accumulate on the vector
engine into an fp16 SBUF tile, ONE 512-wide fp16 ones-matmul per head does
the cross-partition sum, reciprocal_approx_fast computes 1/den. PSUM:
scores 2x2 banks + attention-out 1 + rowsum 1 + (qkv 2 | o_proj 2) = 8.
out_cols written fp16 (host casts back; halves output traffic).

Matmul operands are fp16 (1 cyc/row) with fp32 PSUM accumulation.
LDWEIGHTS is fully hidden by the PE's reorder window; back-to-back MMs
issue at exactly N cycles. The chip's power manager holds the PE at
K=13/16 of 2.4GHz (~1.95GHz, 263ns per 512-wide MM) for any sustained
stream — brief 216ns/2.4GHz bursts appear only in the first ~30us — so
the streaming floor is 1.196M cycles ~= 613us. Measured 645-651us
(baseline 719.8us): ~15us startup supply + ~9us preamble + ~6us tail +
~5us att/op micro-bubbles. fp8 was ruled out: random-sign contractions
preserve operand quantization error (~2-3% rel for e4m3) with no sqrt(N)
averaging, blowing the 2e-2 budget.
"""

import numpy as np

import concourse.bacc as bacc
import concourse.bass_isa as bass_isa
import concourse.mybir as mybir
import concourse.tile as tile
from concourse.bass_utils import run_bass_kernel_spmd

F32 = mybir.dt.float32

N_CORES = 8
NUM_HEADS = 32
HEAD_DIM = 128
P = 128          # SBUF partitions / PE contraction tile
SQ = 512         # s_q block width (PSUM bank = 512 fp32)
MM_MODE = "f16"  # 'f16' | 'f32' (operand dtype for matmuls)

_CACHE: dict = {}


def _mm_dtype(mode):
    return {"f16": mybir.dt.float16, "f32": F32}[mode]


def build(S, H, mode=MM_MODE):
    MD = _mm_dtype(mode)
    hpc = NUM_HEADS // N_CORES          # heads per core
    dpc = hpc * HEAD_DIM                # per-core slice of the hidden dim
    n_ht = H // P                       # contraction tiles for QKV/o_proj
    n_qk = 2 * dpc // P                 # q+k output tiles
    n_sq = S // SQ                      # s_q blocks
    n_st = S // P                       # s_k tiles
    scale = 1.0 / np.sqrt(np.float32(HEAD_DIM))
    s_half = S // 2
    sb_per_half = s_half // SQ
    diag_per_b = SQ // P                # diagonal k-tiles per s_q block

    nc = bacc.Bacc("TRN2", target_bir_lowering=False, debug=False,
                   num_devices=N_CORES)

    # inputs are host-retiled partition-major so every DMA chunk is one
    # contiguous multi-KB run per partition: naive [H, S]-style layouts
    # expand to 1KB/256B descriptors (1024-4096 per chunk) and the sync
    # engine pays ~4-8ns per descriptor — x0's load alone burned ~40us of
    # engine-serialized descriptor pushing
    x5 = nc.dram_tensor("x5", [P, S // SQ, n_ht, SQ], MD,
                        kind="ExternalInput")
    wqk5 = nc.dram_tensor("wqk5", [P, n_qk, n_ht, P], MD,
                          kind="ExternalInput")
    wv5 = nc.dram_tensor("wv5", [P, n_ht, dpc], MD, kind="ExternalInput")
    tri = nc.dram_tensor("tri", [P, P], F32, kind="ExternalInput")
    wo5 = nc.dram_tensor("wo5", [P, n_ht, dpc], MD, kind="ExternalInput")
    out_cols = nc.dram_tensor("out_cols", [S, dpc], MD,
                              kind="ExternalOutput")

    # AllGather in head-pair chunks: gat[b][pp] holds local heads
    # {2pp, 2pp+1} for s_q block b; ct[b][pp] gathers those pairs from all
    # ranks. o_proj consumes them against host-permuted w_o rows.
    gat_b = [[nc.dram_tensor(f"gat_{b}_{pp}", [dpc // 2, SQ], MD)
              for pp in range(2)] for b in range(n_sq)]
    ct_b = [[nc.dram_tensor(f"ct_{b}_{pp}", [H // 2, SQ], MD,
                            addr_space="Shared") for pp in range(2)]
            for b in range(n_sq)]



    with tile.TileContext(nc) as tc:
        with (
            tc.tile_pool(name="consts", bufs=1) as cpool,
            tc.tile_pool(name="span", bufs=1) as span,
            tc.tile_pool(name="qkv_wv", bufs=1) as wvpool,
            tc.tile_pool(name="at_exp", bufs=3) as epool,
            tc.tile_pool(name="at_out", bufs=2) as opool,
            tc.tile_pool(name="at_r", bufs=2) as rpool,
        ):
            ones_s = cpool.tile([P, P], F32, tag="ones_s")
            nc.gpsimd.memset(ones_s[:], 1.0)
            ones_f = cpool.tile([P, P], MD, tag="ones_f")
            nc.vector.tensor_copy(ones_f[:], ones_s[:])
            tri_t = cpool.tile([P, P], F32, tag="tri")

            # v ([s_k, d] natural, all heads) and q/k (transposed, all heads)
            # live in SBUF for the whole kernel; QKV evictions write them
            # directly (no DRAM bounce)
            v_sb = span.tile([P, n_st, dpc], MD, tag="v")
            qk_all = span.tile([P, n_qk, S], MD, tag="qk")
            wv_sb = wvpool.tile([P, n_ht, dpc], MD, tag="wv")

            # x arrives in 8-t-tile chunks: batching descriptors keeps the
            # sync queue's ~0.7us/descriptor issue rate off the critical
            # path.
            XCH = 8

            NXC = 32 // XCH     # chunk tiles per sb block

            # each 8-t chunk is its OWN tile: readers then depend on just
            # their chunk's DMA, not the whole 4MB block (tile reads wait
            # ALL writers of the tile, so a monolithic x tile serialized
            # the first matmul behind the full block's arrival)
            def load_x(half, xpool):
                xq = [[xpool.tile([P, XCH, SQ], MD, tag="x", bufs=2 * NXC,
                                  name="x_tile") for _ in range(NXC)]
                      for _ in range(sb_per_half)]
                for sb in range(sb_per_half):
                    for c in range(NXC):
                        cb = half * sb_per_half + sb
                        nc.sync.dma_start(
                            xq[sb][c][:],
                            x5.ap()[:, cb, c * XCH:(c + 1) * XCH, :])
                return xq

            def xsl(xq_sb, t):
                ch = n_ht // len(xq_sb)
                return xq_sb[t // ch][:, t % ch, :]

            def wsl(w_ot, t):
                if isinstance(w_ot, list):
                    ch = n_ht // len(w_ot)
                    return w_ot[t // ch][:, t % ch, :]
                return w_ot[:, t, :]

            # q/k projection for the given sb blocks of one half, output
            # transposed into resident qk_all. ot_order lets the second
            # half1 pass run reversed so it reuses the 3 still-cached w
            # tiles instead of reloading them.
            def qkv_qk(half, xq, w_tiles, wpool, pspool, sbs,
                       ot_order=None):
                for ot in (ot_order or range(n_qk)):
                    if ot not in w_tiles:
                        w_tiles[ot] = wpool.tile([P, n_ht, P], MD,
                                                 tag="w", name="w_tile")
                        nc.sync.dma_start(
                            w_tiles[ot][:], wqk5.ap()[:, ot, :, :])
                    w_ot = w_tiles[ot]
                    # wv is first needed ~134us in (the v phase); issuing it
                    # at ot==7 keeps its 4MB out of the bandwidth-starved
                    # startup window
                    if half == 0 and ot == 7:
                        nc.sync.dma_start(wv_sb[:], wv5.ap()[:, :, :])
                    if half == 0:
                        # t-outer with both quarters' psums open: the first
                        # ot pass consumes x chunks in DMA-arrival order
                        # instead of draining quarter 0 first, which halves
                        # the startup underrun while x is still streaming
                        # in. Needs 4 psum bufs, so half1 (2 bufs, x fully
                        # prefetched behind att(0)/att(1)) keeps the
                        # sb-inner form.
                        ps_q = [pspool.tile([P, SQ], F32, tag="qkv",
                                            name=f"ps_q{sb}")
                                for sb in sbs]
                        for t in range(n_ht):
                            for si, sb in enumerate(sbs):
                                nc.tensor.matmul(
                                    ps_q[si][:],
                                    wsl(w_ot, t),
                                    xsl(xq[sb], t),
                                    start=(t == 0), stop=(t == n_ht - 1))
                        for si, sb in enumerate(sbs):
                            # fold the softmax scale into q at eviction
                            mul = scale if ot < dpc // P else 1.0
                            lo = half * s_half + sb * SQ
                            nc.scalar.mul(qk_all[:, ot, lo:lo + SQ],
                                          ps_q[si][:], mul)
                    else:
                        for sb in sbs:
                            ps = pspool.tile([P, SQ], F32, tag="qkv")
                            for t in range(n_ht):
                                nc.tensor.matmul(
                                    ps[:],
                                    wsl(w_ot, t),
                                    xsl(xq[sb], t),
                                    start=(t == 0), stop=(t == n_ht - 1))
                            mul = scale if ot < dpc // P else 1.0
                            lo = half * s_half + sb * SQ
                            nc.scalar.mul(qk_all[:, ot, lo:lo + SQ],
                                          ps[:], mul)

            # v projection for the given local s-tiles of one half:
            # psum [s=128, dpc] accumulated over h-tiles
            def qkv_v(half, xq, pspool, stis):
                for sti in stis:
                    st_g = half * (s_half // P) + sti
                    sb, off = (sti * P) // SQ, (sti * P) % SQ
                    ps_v = pspool.tile([P, dpc], F32, tag="qkv")
                    for t in range(n_ht):
                        xch = n_ht // len(xq[sb])
                        nc.tensor.matmul(
                            ps_v[:],
                            xq[sb][t // xch][:, t % xch, off:off + P],
                            wv_sb[:, t, :],
                            start=(t == 0), stop=(t == n_ht - 1))
                    nc.vector.tensor_copy(v_sb[:, st_g, :], ps_v[:])

            with (
                tc.tile_pool(name="qkv_x0", bufs=1) as xpool0,
                tc.tile_pool(name="qkv_w0", bufs=3) as wpool0,
                tc.tile_pool(name="qkv_ps0", bufs=4, space="PSUM") as psp0,
            ):
                # startup ordering: the t-outer first ot pass consumes
                # (w0 t-chunk, x sb0 chunk, x sb1 chunk) groups in order, so
                # issue the DMAs in exactly that order — the first matmul
                # then waits on 2.25MB, not on all of x + w.
                XC0 = XCH
                NX0 = n_ht // XC0
                w0c = [wpool0.tile([P, XC0, P], MD, tag="w0c",
                                   name="w0c") for _ in range(NX0)]
                w0_tiles = {0: w0c}
                xq0 = [[xpool0.tile([P, XC0, SQ], MD, tag="x",
                                    bufs=2 * NX0, name="x_tile")
                        for _ in range(NX0)] for _ in range(sb_per_half)]
                # sb1's chunks ride the scalar engine's separate hardware
                # DMA queue: chunk pairs arrive in parallel and the
                # descriptor issue cost is split across two engines
                for c in range(NX0):
                    nc.sync.dma_start(
                        w0c[c][:], wqk5.ap()[:, 0, c * XC0:(c + 1) * XC0, :])
                    nc.sync.dma_start(
                        xq0[0][c][:], x5.ap()[:, 0, c * XC0:(c + 1) * XC0, :])
                    nc.scalar.dma_start(
                        xq0[1][c][:], x5.ap()[:, 1, c * XC0:(c + 1) * XC0, :])
                # tri is first read at att(0), ~215us in — keep its 64KB of
                # 512B-packet traffic out of the startup window
                nc.scalar.dma_start(tri_t[:], tri.ap()[:, :])
                qkv_qk(0, xq0, w0_tiles, wpool0, psp0, [0, 1])
                qkv_v(0, xq0, psp0, range(s_half // P))



            # attention PSUM pools: pss 2x2 banks + out 1 + row 1 = 6 banks,
            # leaving 2 for the half1 QKV pool / later the o_proj pool
            with tc.tile_pool(name="at_ps", bufs=1, space="PSUM") as aps:

                # ======== attention for s_q block b (4 local heads) ========
                # scores are computed transposed (scoresT[k, q]) so the PE
                # contraction dim sits on partitions for every matmul.
                # Diagonal tiles are narrowed to columns [128j, 512) and get
                # the triangular mask strip added in place. Tiles are
                # processed in slots of two; exp of slot k runs while the PE
                # streams slot k+1's scores, and av/rowsum of slot k follow —
                # steady state has no PE bubble.
                def att_block(b):
                    q_lo = b * SQ
                    n_full = b * diag_per_b
                    slots = []
                    for g in range(n_full // 2):
                        slots.append([(2 * g, 0), (2 * g + 1, 0)])
                    for g in range(diag_per_b // 2):
                        slots.append([(n_full + 2 * g, 2 * g),
                                      (n_full + 2 * g + 1, 2 * g + 1)])
                    n_mm = n_full + diag_per_b
                    for h in range(hpc):
                        ps_o = aps.tile([P, SQ], F32, tag="out",
                                        name="ps_o")
                        # softmax denominator: accumulate exp tiles on the
                        # vector engine (fp16 adds into an SBUF fp32
                        # accumulator), cross-partition-sum via one
                        # ones-matmul per head
                        acc = rpool.tile([P, SQ], MD,
                                         tag="acc", name="acc")
                        mi = 0

                        def emit_av(pend):
                            nonlocal mi
                            slot, ex2 = pend
                            for ii, (t, j) in enumerate(slot):
                                lo = P * j
                                first, last = mi == 0, mi == n_mm - 1
                                nc.tensor.matmul(
                                    ps_o[:, lo:SQ],
                                    v_sb[:, t, h * P:(h + 1) * P],
                                    ex2[:, ii, lo:SQ],
                                    start=first, stop=last,
                                    skip_group_check=True)
                                if first:
                                    # first tile is always full-width
                                    nc.vector.tensor_copy(
                                        acc[:], ex2[:, ii, :])
                                else:
                                    nc.vector.tensor_add(
                                        acc[:, lo:SQ], acc[:, lo:SQ],
                                        ex2[:, ii, lo:SQ])
                                mi += 1

                        # av/rowsum lag scores by TWO slots: ps2 is freed by
                        # the exp read (not by av), so depth-2 costs no extra
                        # PSUM and the scores->exp->av chain latency is fully
                        # hidden even at block starts.
                        pend = []
                        for slot in slots:
                            ps2 = aps.tile([P, 2, SQ], F32, tag="pss",
                                           bufs=2, name="ps2")
                            ex2 = epool.tile([P, 2, SQ], MD, tag="exp",
                                             name="ex2")
                            is_diag = slot[0][0] >= n_full
                            for ii, (t, j) in enumerate(slot):
                                lo = P * j
                                nc.tensor.matmul(
                                    ps2[:, ii, lo:SQ],
                                    qk_all[:, hpc + h, t * P:(t + 1) * P],
                                    qk_all[:, h, q_lo + lo:q_lo + SQ],
                                    start=True, stop=True)
                                if is_diag:
                                    nc.vector.tensor_add(
                                        ps2[:, ii, lo:lo + P],
                                        ps2[:, ii, lo:lo + P], tri_t[:])
                            if is_diag:
                                for ii, (t, j) in enumerate(slot):
                                    lo = P * j
                                    nc.scalar.activation(
                                        ex2[:, ii, lo:SQ], ps2[:, ii, lo:SQ],
                                        mybir.ActivationFunctionType.Exp)
                            else:
                                nc.scalar.activation(
                                    ex2[:, :, :], ps2[:, :, :],
                                    mybir.ActivationFunctionType.Exp)
                            pend.append((slot, ex2))
                            if len(pend) > 2:
                                emit_av(pend.pop(0))
                        for p in pend:
                            emit_av(p)

                        # cross-partition sum of the exp accumulator in ONE
                        # 512-wide fp16 ones-matmul (f32r ran as fp32-HIGH,
                        # ~694ns + exposed fp32 LDW, ~8us of PE across the
                        # 16 head-blocks; fp16 accumulation of the <=2048-
                        # term denominator adds only ~1e-3 rel error)
                        ps_row = aps.tile([P, SQ], F32, tag="row",
                                          name="ps_row")
                        nc.tensor.matmul(
                            ps_row[:], ones_f[:], acc[:],
                            start=True, stop=True)
                        recip = rpool.tile([P, SQ], F32, tag="recip",
                                           name="recip")
                        nc.vector.reciprocal_approx_fast(recip[:], ps_row[:])
                        ob = opool.tile([P, SQ], MD, tag="ob", name="ob")
                        nc.vector.tensor_mul(ob[:], ps_o[:], recip[:])
                        nc.sync.dma_start(
                            gat_b[b][h // 2].ap()[(h % 2) * P:
                                                  (h % 2 + 1) * P, :], ob[:])
                        if h % 2 == 1:
                            nc.gpsimd.collective_compute(
                                "AllGather", mybir.AluOpType.bypass,
                                replica_groups=[list(range(N_CORES))],
                                ins=[gat_b[b][h // 2].ap().opt()],
                                outs=[ct_b[b][h // 2].ap().opt()])

                # att(0)+att(1) fit between the QKV halves: they need only
                # first-half k/v/q, and their ~26us of PE work hides the
                # half1 x/w prefetch (issued first, below — the x1 DMAs
                # have no dependencies and stream during qkv0's tail).
                # att(2)/att(3) interleave INTO half1 right after the k/v
                # tiles they need exist, so their gathers fire at ~370/~505
                # instead of after the whole half — the serial CC-core
                # chain (~20-40us per 2MB op) then finishes well before the
                # last o_proj block needs its ct.
                with (
                    tc.tile_pool(name="qkv_x1", bufs=1) as xpool1,
                    tc.tile_pool(name="qkv_w1", bufs=3) as wpool1,
                    tc.tile_pool(name="qkv_ps1", bufs=2,
                                 space="PSUM") as psp1,
                ):
                    w1_tiles = {0: wpool1.tile([P, n_ht, P], MD, tag="w",
                                               name="w_tile")}
                    nc.sync.dma_start(w1_tiles[0][:],
                                      wqk5.ap()[:, 0, :, :])
                    xq1 = load_x(1, xpool1)
                    att_block(0)
                    att_block(1)
                    qkv_qk(1, xq1, w1_tiles, wpool1, psp1, [0])
                    qkv_v(1, xq1, psp1, range(0, s_half // P // 2))
                    att_block(2)
                    # reversed ot: w7/w6/w5 are still live in the 3-buf
                    # pool from the sb0 pass — no reload for them, and the
                    # rotation's reuse-dependencies naturally pace the
                    # remaining reloads one ot ahead of consumption
                    w1b = {ot: w1_tiles[ot] for ot in (5, 6, 7)}
                    qkv_qk(1, xq1, w1b, wpool1, psp1, [1],
                           ot_order=range(n_qk - 1, -1, -1))
                    qkv_v(1, xq1, psp1, range(s_half // P // 2,
                                              s_half // P))
                    att_block(3)

                with (
                    tc.tile_pool(name="op_ct", bufs=32) as ctpool,
                    tc.tile_pool(name="op_w", bufs=1) as owpool,
                    tc.tile_pool(name="op_stage", bufs=4) as ospool,
                    tc.tile_pool(name="op_ps", bufs=2, space="PSUM") as opsp,
                ):
                    # wo in 4 chunk tiles interleaved with ct staging
                    # (all of it waits the last x1 read ~465us; op(0)'s
                    # first matmuls need only woc0 + ct0-pp0, so interleave
                    # in consumption order instead of serializing 4MB of wo
                    # ahead of 8MB of ct)
                    wo_sb = [owpool.tile([P, XCH, dpc], MD, tag="wo",
                                         bufs=NXC, name="woc")
                             for _ in range(NXC)]
                    def prefetch_ct(b):
                        # 2 t-tiles per descriptor: halves the post-gather
                        # issue serialization on the sync queue
                        cts = []
                        for pp in range(2):
                            ct_t = ct_b[b][pp].ap().rearrange(
                                "(t p) s -> p t s", p=P)
                            for t2 in range(n_ht // 4):
                                c_t = ctpool.tile([P, 2, SQ], MD, tag="ct")
                                nc.sync.dma_start(
                                    c_t[:], ct_t[:, 2 * t2:2 * t2 + 2, :])
                                cts.append(c_t)
                        return cts

                    def emit_oproj(b, cts):
                        # contraction split into pp phases (w_o rows are
                        # host-permuted so tiles 0-15 = pair0 of all ranks,
                        # 16-31 = pair1): a block can start on its pp0
                        # gather while pp1's is still in flight — op(3)'s
                        # pp1 lands only ~8us before it's needed
                        hht = n_ht // 2
                        for sp in range(SQ // P // 2):
                            pss = [opsp.tile([P, dpc], F32, tag="op",
                                             name="op_ps")
                                   for _ in range(2)]
                            for ph in range(2):
                                for si in range(2):
                                    st = 2 * sp + si
                                    for t in range(ph * hht,
                                                   (ph + 1) * hht):
                                        nc.tensor.matmul(
                                            pss[si][:],
                                            cts[t // 2][:, t % 2,
                                                        st * P:(st + 1) * P],
                                            wo_sb[t // XCH][:, t % XCH, :],
                                            start=(t == 0),
                                            stop=(t == n_ht - 1))
                                    if ph == 1:
                                        ob = ospool.tile(
                                            [P, dpc], MD, tag="ostage",
                                            name="ostage")
                                        nc.scalar.copy(ob[:], pss[si][:])
                                        nc.sync.dma_start(
                                            out_cols.ap()[
                                                b * SQ + st * P:
                                                b * SQ + (st + 1) * P, :],
                                            ob[:])

                    # ct staging physically reuses the x1/w1 SBUF region,
                    # so its descriptors can start only once the last x
                    # read (v[12-15], ~465us) completes — by then gathers
                    # g0..g2 are long done and the 32-buf pool double-
                    # buffers: ct0+ct1 stage back-to-back before op(0)'s
                    # PE arrival, ct2/ct3 into the halves freed by
                    # op(0)/op(1).
                    def wo_chunk(c):
                        nc.sync.dma_start(
                            wo_sb[c][:],
                            wo5.ap()[:, c * XCH:(c + 1) * XCH, :])

                    cts0 = prefetch_ct(0)
                    wo_chunk(0)
                    wo_chunk(1)
                    cts1 = prefetch_ct(1)
                    wo_chunk(2)
                    wo_chunk(3)
                    emit_oproj(0, cts0)
                    cts2 = prefetch_ct(2)
                    emit_oproj(1, cts1)
                    cts3 = prefetch_ct(3)
                    emit_oproj(2, cts2)
                    emit_oproj(3, cts3)

    nc.compile()
    return nc


def _tile5(a2d, inner):
    """[H, O] -> partition-major tiles [P, O//inner, H//P, inner]:
    per (partition, outer-block) the (t, inner) plane is contiguous, so
    chunk DMAs become one multi-KB descriptor per partition."""
    H, O = a2d.shape
    a = a2d.reshape(H // P, P, O // inner, inner)
    return np.ascontiguousarray(a.transpose(1, 2, 0, 3))


def make_in_maps(hidden_states, attention_mask, w_pack, w_o):
    B, S, H = hidden_states.shape
    hpc = NUM_HEADS // N_CORES
    dpc = hpc * HEAD_DIM
    np_md = mybir.dt.np(_mm_dtype(MM_MODE))
    xT = hidden_states[0].T.astype(np_md)
    x5 = _tile5(xT, SQ)
    # triangular mask strip for diagonal score tiles (scoresT layout:
    # rows=s_k, cols=s_q; masked where k > q -> strictly lower triangle)
    tri = np.tril(np.full((P, P), np.finfo(np.float32).min,
                          dtype=np.float32), k=-1)
    # w_o rows permuted to match the head-pair AllGather layout:
    # [pp][rank][head-in-pair] blocks of 128
    perm = np.concatenate(
        [np.arange(128 * (4 * r + 2 * pp + hh),
                   128 * (4 * r + 2 * pp + hh) + 128)
         for pp in (0, 1) for r in range(N_CORES) for hh in (0, 1)])
    in_maps = []
    for c in range(N_CORES):
        sl = slice(c * dpc, (c + 1) * dpc)
        wqk_c = np.concatenate(
            [w_pack[0 * H:1 * H][sl], w_pack[1 * H:2 * H][sl]], axis=0)
        woT_c = w_o[sl].T[perm]
        in_maps.append({
            "x5": x5,
            "wqk5": _tile5(wqk_c.T.astype(np_md), P),
            "wv5": _tile5(w_pack[2 * H:3 * H][sl].T.astype(np_md), dpc),
            "tri": tri,
            "wo5": _tile5(woT_c.astype(np_md), dpc),
        })
    return in_maps, tri


def kernel(hidden_states, attention_mask, w_pack, w_o):
    B, S, H = hidden_states.shape
    assert B == 1 and H == NUM_HEADS * HEAD_DIM
    assert S % (2 * SQ) == 0

    # the kernel hardcodes the causal structure; verify the mask matches
    mask = np.asarray(np.broadcast_to(attention_mask, (1, 1, S, S))[0, 0],
                      dtype=np.float32)
    assert np.all(np.tril(mask) == 0.0), "mask must be causal"
    assert np.all(mask[np.triu_indices(S, 1)] <= -1e30), "mask must be causal"

    in_maps, _ = make_in_maps(hidden_states, attention_mask, w_pack, w_o)

    key = (S, H, MM_MODE)
    if key not in _CACHE:
        _CACHE[key] = build(S, H, MM_MODE)
    nc = _CACHE[key]

    res = run_bass_kernel_spmd(nc, in_maps, core_ids=list(range(N_CORES)))
    out = np.concatenate(
        [res.results[c]["out_cols"] for c in range(N_CORES)], axis=1)
    return out.reshape(1, S, H).astype(np.float32)
